# revision 14
# baseline (speedup 1.0000x reference)
"""Bilateral filter (7x7, sigma_color=0.1) Trainium2 Bass kernel.

Strategy:
  - Host: zero-pad image, cast to fp16, shard 4(H) x 2(W) across 8 cores,
    pre-expand each core's shard into 7 pre-shifted "strip stacks":
      partition p = jy*18 + r  (7 row-shift strips x 18 rows = 126 partitions)
      ST[b, o, p, c*xw+x] = Ipad[c, y0 + jy + r, o + x]   (x-shift o baked in)
      CC[b, p, c*xw+x]    = Ipad[c, y0 + 3 + r, 3 + x]    (center, replicated)
  - Device per (block b, x-shift o in 0..6), elementwise fp16 (DVE 2x mode):
      diff   = ST[b,o] - CC      (DVE, one [126, 3*xw] unit-stride op)
      sq     = diff^2            (ACT Square, one [126, 3*xw] op)
      d1     = sq0 + sq1         (DVE)
      D      = d1 + sq2          (DVE)
      F      = exp(-50*D + bias_p) (ACT; bias_p = ln(norm_color*g[jy,o]) per strip)
      V_c    = F * ST_c          (DVE x3) -> packed tile vf = [V0 V1 V2 F]
      accumulate over (jy, o) on TensorE: 5 matmuls N=512,
        psum[18, 2560] += collapse[126,18].T @ vf   (fp32 accumulation)
  - Finalize per 7-block group (emitted inline): evacuate psum into strip-
    gathered [126, xw] tiles, reciprocal(den) on DVE, num*rec on GPSIMD.
  - Duplicate LDWEIGHTS of the stationary collapse matrix are pruned from the
    BIR before walrus compiles it (PE weights persist across matmuls).
"""

import math

import numpy as np

import concourse.bass as bass
import concourse.bacc as bacc
import concourse.mybir as mybir
from concourse.tile import TileContext

F16 = np.float16
F32 = np.float32

# problem constants
H, W, C = 720, 1280, 3
K = 7
PAD = 3
SIGMA_COLOR = 0.1
NORM_COLOR = 1.0 / (2.0 * math.pi * SIGMA_COLOR**2)
EXP_SCALE = -1.0 / (2.0 * SIGMA_COLOR**2)  # -50.0

# sharding / tiling constants
HSH, WSH = 4, 2          # core grid (8 cores)
RB = 18                  # output rows per block
JY = 7                   # row-shift strips
P = JY * RB              # 126 partitions used
XW = W // WSH            # 640 output cols per core
N_CORES = 8
MMN = 512                # matmul free-dim max (one PSUM bank)


def _alu(name):
    return getattr(mybir.AluOpType, name)


def build_nc(nb: int, xw: int = XW):
    """Build the Bass program for one core processing nb blocks of RB rows x xw cols."""
    dt = mybir.dt
    nc = bacc.Bacc("TRN2", debug=False)

    ST = nc.dram_tensor("ST", [nb, K, P, C * xw], dt.float16, kind="ExternalInput")
    CC = nc.dram_tensor("CC", [nb, P, C * xw], dt.float16, kind="ExternalInput")
    CP = nc.dram_tensor("CP", [nb, P, 2 * C * xw], dt.float16, kind="ExternalInput")
    BI = nc.dram_tensor("BI", [128, 8], dt.float32, kind="ExternalInput")
    CL = nc.dram_tensor("CL", [P, RB], dt.float16, kind="ExternalInput")
    OUT = nc.dram_tensor("OUT", [C, nb * RB, xw], dt.float32, kind="ExternalOutput")

    fw = (C + 1) * xw  # packed vf width (V0 V1 V2 F)
    assert fw % MMN == 0
    n_mm = fw // MMN
    half = fw // 2

    n_grp = (nb + 6) // 7
    grp_rows = [min(7, nb - 7 * g) * RB for g in range(n_grp)]
    total = nb * K

    with TileContext(nc) as tc:
        with (
            tc.tile_pool(name="singles", bufs=1) as psingle,
            tc.tile_pool(name="stack", bufs=3) as pstack,
            tc.tile_pool(name="cstack", bufs=2) as pcstack,
            tc.tile_pool(name="work", bufs=2) as pwork,
            tc.tile_pool(name="psum", bufs=1, space="PSUM") as ppsum,
            tc.tile_pool(name="stage", bufs=2) as pstage,
            tc.tile_pool(name="gather", bufs=1) as pgather,
            tc.tile_pool(name="fin", bufs=2) as pfin,
        ):
            bi = psingle.tile([128, 8], dt.float32, tag="bias")
            nc.sync.dma_start(bi[:, :], BI[:, :])
            cl = psingle.tile([P, RB], dt.float16, tag="coll")
            nc.sync.dma_start(cl[:, :], CL[:, :])

            gat = {}
            for f_i in range(4):  # 0..2 = num channels, 3 = den
                for g in range(n_grp):
                    gat[(f_i, g)] = pgather.tile(
                        [126, xw], dt.float32, tag=f"gat{f_i}_{g}", name=f"gat{f_i}_{g}"
                    )

            # psum piece table: cuts at every bank (512) and field (640) boundary
            cuts = sorted(set(range(0, fw + 1, MMN)) | set(range(0, fw + 1, xw)))
            pieces = [(cuts[i], cuts[i + 1]) for i in range(len(cuts) - 1)]

            def finalize(g):
                rg = grp_rows[g]
                rec = pfin.tile([126, xw], dt.float32, tag="rec", name="rec")
                nc.vector.reciprocal(rec[0:rg, :], gat[(3, g)][0:rg, :])
                for c in range(C):
                    ot = pfin.tile([126, xw], dt.float32, tag="ot", name="ot")
                    nc.gpsimd.tensor_tensor(
                        ot[0:rg, :], gat[(c, g)][0:rg, :], rec[0:rg, :], _alu("mult")
                    )
                    nc.sync.dma_start(OUT[c, g * 126 : g * 126 + rg, :], ot[0:rg, :])

            w2 = 2 * xw  # 1280: per-(c) pair segment width
            for b in range(nb):
                cc = pcstack.tile([P, C * xw], dt.float16, tag="cc", name="cc")
                nc.sync.dma_start(cc[:, :], CC[b])
                cp = pcstack.tile([P, 2 * C * xw], dt.float16, tag="cp", name="cp")
                nc.sync.dma_start(cp[:, :], CP[b])
                pp = ppsum.tile([RB, fw], dt.float32, tag="pp", name="pp")

                # --- three o-pairs in (c, o, x) packed layout ---
                for pi in range(3):
                    o0 = 2 * pi
                    stp = pstack.tile([P, 2 * C * xw], dt.float16, tag="stp", name="stp")
                    ov = stp[:].rearrange("p (c o x) -> p c o x", o=2, x=xw)
                    iv = ST[b, o0 : o0 + 2].rearrange("o p (c x) -> p c o x", x=xw)
                    nc.sync.dma_start(ov, iv)

                    dfp = pwork.tile([P, 2 * C * xw], dt.float16, tag="dfp", name="dfp")
                    nc.vector.tensor_tensor(dfp[:, :], stp[:, :], cp[:, :], _alu("subtract"))
                    sqp = pwork.tile([P, 2 * C * xw], dt.float16, tag="sqp", name="sqp")
                    nc.scalar.activation(
                        sqp[:, :], dfp[:, :], mybir.ActivationFunctionType.Square
                    )
                    d1p = pwork.tile([P, w2], dt.float16, tag="d1p", name="d1p")
                    nc.vector.tensor_tensor(
                        d1p[:], sqp[:, 0:w2], sqp[:, w2 : 2 * w2], _alu("add")
                    )
                    d2p = pwork.tile([P, w2], dt.float16, tag="d2p", name="d2p")
                    nc.vector.tensor_tensor(
                        d2p[:], d1p[:], sqp[:, 2 * w2 : 3 * w2], _alu("add")
                    )

                    # vfp layout (c, o, x): V_c at c*w2 + oh*xw, F at 3*w2 + oh*xw
                    vfp = pwork.tile(
                        [P, (C + 1) * w2], dt.float16, tag="vfp", name="vfp", bufs=3
                    )
                    for oh in range(2):
                        nc.scalar.activation(
                            vfp[:, C * w2 + oh * xw : C * w2 + (oh + 1) * xw],
                            d2p[:, oh * xw : (oh + 1) * xw],
                            mybir.ActivationFunctionType.Exp,
                            bias=bi[0:P, o0 + oh : o0 + oh + 1],
                            scale=float(EXP_SCALE),
                        )
                    for c in range(C):
                        nc.vector.tensor_tensor(
                            vfp[:, c * w2 : (c + 1) * w2],
                            vfp[:, C * w2 : (C + 1) * w2],
                            stp[:, c * w2 : (c + 1) * w2],
                            _alu("mult"),
                        )

                    # accumulate both o-halves into psum; pieces are cut at every
                    # bank and field boundary, emitted in ascending psum-column
                    # order. start=True only on round 0 (pi==0, oh==0) pieces
                    # that begin at a bank boundary: that matmul clears its
                    # whole bank, and same-round pieces later in the bank
                    # overwrite-where-unwritten.
                    for oh in range(2):
                        for (a, e) in pieces:
                            f_i = a // xw
                            ra = f_i * w2 + oh * xw + (a - f_i * xw)
                            st_ = pi == 0 and oh == 0 and a % MMN == 0
                            nc.tensor.matmul(
                                pp[:, a:e],
                                cl[:, :],
                                vfp[:, ra : ra + (e - a)],
                                start=st_,
                                stop=False,
                            )

                # --- single o = 6 (old layout, 1:1 psum columns) ---
                st6 = pstack.tile([P, C * xw], dt.float16, tag="stp", name="st6")
                nc.sync.dma_start(st6[:, :], ST[b, K - 1])
                df6 = pwork.tile([P, C * xw], dt.float16, tag="dfp", name="df6")
                nc.vector.tensor_tensor(df6[:, :], st6[:, :], cc[:, :], _alu("subtract"))
                sq6 = pwork.tile([P, C * xw], dt.float16, tag="sqp", name="sq6")
                nc.scalar.activation(
                    sq6[:, :], df6[:, :], mybir.ActivationFunctionType.Square
                )
                d16 = pwork.tile([P, xw], dt.float16, tag="d1p", name="d16")
                nc.vector.tensor_tensor(
                    d16[:], sq6[:, 0:xw], sq6[:, xw : 2 * xw], _alu("add")
                )
                d26 = pwork.tile([P, xw], dt.float16, tag="d2p", name="d26")
                nc.vector.tensor_tensor(
                    d26[:], d16[:], sq6[:, 2 * xw : 3 * xw], _alu("add")
                )
                vf6 = pwork.tile([P, fw], dt.float16, tag="vfp", name="vf6", bufs=3)
                nc.scalar.activation(
                    vf6[:, C * xw : fw],
                    d26[:],
                    mybir.ActivationFunctionType.Exp,
                    bias=bi[0:P, K - 1 : K],
                    scale=float(EXP_SCALE),
                )
                for c in range(C):
                    nc.vector.tensor_tensor(
                        vf6[:, c * xw : (c + 1) * xw],
                        vf6[:, C * xw : fw],
                        st6[:, c * xw : (c + 1) * xw],
                        _alu("mult"),
                    )
                for m in range(n_mm):
                    nc.tensor.matmul(
                        pp[:, m * MMN : (m + 1) * MMN],
                        cl[:, :],
                        vf6[:, m * MMN : (m + 1) * MMN],
                        start=False,
                        stop=True,
                    )

                # evacuate psum -> staging -> gather tiles
                g, idx = b // 7, b % 7
                stga = pstage.tile([RB, half], dt.float32, tag="stga", name="stga")
                nc.scalar.copy(stga[:], pp[:, 0:half])
                stgb = pstage.tile([RB, half], dt.float32, tag="stgb", name="stgb")
                nc.scalar.copy(stgb[:], pp[:, half:fw])
                rows = slice(idx * RB, (idx + 1) * RB)
                nc.sync.dma_start(gat[(0, g)][rows, :], stga[:, 0:xw])
                nc.sync.dma_start(gat[(1, g)][rows, :], stga[:, xw : 2 * xw])
                nc.sync.dma_start(gat[(2, g)][rows, :], stgb[:, 0:xw])
                nc.sync.dma_start(gat[(3, g)][rows, :], stgb[:, xw : 2 * xw])
                if b == 7 * g + 6 or b == nb - 1:
                    finalize(g)

    nc.compile()
    return nc


def host_prepare(I: np.ndarray, gw49: np.ndarray):
    """I: (1, C, Him, Wim) fp32. Returns in_maps for 8 cores + assembly info."""
    _, c_, him, wim = I.shape
    assert c_ == C
    nb = him // (HSH * RB)
    xw = wim // WSH
    rs = nb * RB  # rows per core

    Ip = np.zeros((C, him + 2 * PAD, wim + 2 * PAD), dtype=F32)
    Ip[:, PAD : PAD + him, PAD : PAD + wim] = I[0]
    Ib = Ip.astype(F16)

    # bias + collapse (shared across cores)
    bias = np.zeros((128, 8), dtype=F32)
    gw7 = gw49.reshape(K, K).astype(np.float64)
    for p in range(P):
        jy = p // RB
        bias[p, :K] = np.log(NORM_COLOR * gw7[jy, :]).astype(F32)
    coll = np.zeros((P, RB), dtype=F16)
    for p in range(P):
        coll[p, p % RB] = 1.0

    in_maps = []
    for i in range(N_CORES):
        hi, wi = i // WSH, i % WSH
        sh = Ib[:, rs * hi : rs * hi + rs + 2 * PAD, xw * wi : xw * wi + xw + 2 * PAD]
        s0, s1, s2 = sh.strides
        # ST[b, o, (jy, r), c, x] = sh[c, b*RB + jy + r, o + x]
        w1 = np.lib.stride_tricks.as_strided(
            sh,
            shape=(C, nb, K, JY, RB, xw),
            strides=(s0, RB * s1, s2, s1, s1, s2),
        )
        STa = np.ascontiguousarray(w1.transpose(1, 2, 3, 4, 0, 5)).reshape(
            nb, K, P, C * xw
        )
        shc = sh[:, PAD:, PAD:]
        w3 = np.lib.stride_tricks.as_strided(
            shc, shape=(C, nb, JY, RB, xw), strides=(s0, RB * s1, 0, s1, s2)
        )
        CCa = np.ascontiguousarray(w3.transpose(1, 2, 3, 0, 4)).reshape(
            nb, P, C * xw
        )
        CPa = np.ascontiguousarray(
            np.broadcast_to(
                CCa.reshape(nb, P, C, 1, xw), (nb, P, C, 2, xw)
            )
        ).reshape(nb, P, 2 * C * xw)
        in_maps.append({"ST": STa, "CC": CCa, "CP": CPa, "BI": bias, "CL": coll})
    return in_maps, nb, xw, rs


def assemble(results, him, wim, rs, xw):
    out = np.empty((1, C, him, wim), dtype=F32)
    for i in range(N_CORES):
        hi, wi = i // WSH, i % WSH
        out[0, :, rs * hi : rs * hi + rs, xw * wi : xw * wi + xw] = results[i]["OUT"]
    return out


def _numpy_fallback(I, g):
    """Exact reference computation on host (used only if g is not spatially constant)."""
    n, c, h, w = I.shape
    Ipad = np.zeros((n, c, h + 2 * PAD, w + 2 * PAD), dtype=np.float64)
    Ipad[:, :, PAD : PAD + h, PAD : PAD + w] = I
    num = np.zeros((n, c, h, w), dtype=np.float64)
    den = np.zeros((n, h, w), dtype=np.float64)
    g64 = g.astype(np.float64)
    for j in range(K * K):
        dy, dx = j // K, j % K
        S = Ipad[:, :, dy : dy + h, dx : dx + w]
        D = ((S - I.astype(np.float64)) ** 2).sum(axis=1)
        wgt = np.exp(EXP_SCALE * D) * NORM_COLOR * g64[:, j]
        num += wgt[:, None] * S
        den += wgt
    return (num / den[:, None]).astype(F32)


_CACHE = {}
TRACE = False
LAST_EXEC_NS = None
_LDW_PATCHED = False


def _enable_ldw_prune():
    """Drop duplicate LDWEIGHTS of the same stationary lhsT from the BIR before
    walrus compiles it. PE weights persist across matmuls; only loads that carry
    sync conditions (or follow a different weight tensor) are kept."""
    global _LDW_PATCHED
    if _LDW_PATCHED:
        return
    import json as _json
    import concourse.bass_utils as _bu

    _orig = _bu.compile_bir_kernel

    def _prune(bir_json):
        js = _json.loads(bir_json)
        for fn in js.get("functions", []):
            for blk in fn.get("blocks", []):
                insts = blk.get("instructions", [])
                out = []
                last_ldw = None
                for inst in insts:
                    if inst.get("opcode") == "Ldweights":
                        si = inst.get("sync_info") or {}
                        key = _json.dumps(inst.get("ins"), sort_keys=True)
                        if (
                            last_ldw == key
                            and not si.get("on_wait")
                            and not si.get("on_update")
                        ):
                            continue  # duplicate load of identical weights
                        last_ldw = key
                    out.append(inst)
                blk["instructions"] = out
        return _json.dumps(js).encode()

    def _patched(bir_json, tmpdir, neff_name="file.neff"):
        try:
            bir_json = _prune(bir_json)
        except Exception:
            pass
        return _orig(bir_json, tmpdir, neff_name=neff_name)

    _bu.compile_bir_kernel = _patched
    try:
        import concourse.bass2jax as _b2j

        if getattr(_b2j, "compile_bir_kernel", None) is not None:
            _b2j.compile_bir_kernel = _patched
    except Exception:
        pass
    _LDW_PATCHED = True


def kernel(I: np.ndarray, g: np.ndarray) -> np.ndarray:
    global LAST_EXEC_NS
    I = np.asarray(I, dtype=F32)
    g = np.asarray(g)

    gw49 = np.asarray(g[0, :, 0, 0], dtype=F32)
    if not np.array_equal(
        np.asarray(g), np.broadcast_to(np.asarray(g)[:, :, :1, :1], g.shape)
    ):
        return _numpy_fallback(I, g)

    from concourse.bass_utils import run_bass_kernel_spmd

    import os as _os
    if _os.environ.get("BASS_LDW_PRUNE", "1") == "1":
        _enable_ldw_prune()

    in_maps, nb, xw, rs = host_prepare(I, gw49)
    key = (nb, xw)
    if key not in _CACHE:
        _CACHE[key] = build_nc(nb, xw)
    nc = _CACHE[key]
    res = run_bass_kernel_spmd(
        nc, in_maps, core_ids=list(range(N_CORES)), trace=TRACE
    )
    LAST_EXEC_NS = res.exec_time_ns
    return assemble(res.results, I.shape[2], I.shape[3], rs, xw)


if __name__ == "__main__":
    # tiny smoke test in CoreSim: 1 core, small image
    import concourse.bass_interp as bass_interp

    rng = np.random.default_rng(0)
    him, wim = HSH * RB * 2, W  # 2 blocks per core
    I = rng.random((1, C, him, wim), dtype=F32)
    gw49 = np.exp(
        -(np.add.outer(np.arange(-3.0, 4) ** 2, np.arange(-3.0, 4) ** 2)) / 50.0
    ).reshape(-1) * (2 * math.pi * 25.0)
    g = np.tile(gw49.reshape(1, K * K, 1, 1), (1, 1, him, wim)).astype(F32)

    in_maps, nb, xw, rs = host_prepare(I, gw49.astype(F32))
    nc = build_nc(nb, xw)
    sim = bass_interp.CoreSim(nc)
    for k, v in in_maps[0].items():
        sim.tensor(k)[:] = v
    sim.simulate()
    got = np.array(sim.tensor("OUT"))

    exp_full = _numpy_fallback(I, g)
    exp0 = exp_full[0, :, 0:rs, 0:xw]
    err = np.abs(got - exp0)
    print("sim err max:", err.max(), "rel:", err.max() / np.abs(exp0).max())
